# revision 4
# baseline (speedup 1.0000x reference)
"""Multi-head attention on 8 Trainium2 NeuronCores (Bass/Tile).

Problem: B=4, T=2048, DIM=2048, H=16 heads, dk=dv=64.
  q = Q@Wq, k = K@Wk, v = V@Wv  (per head slices)
  out = softmax(q k^T / sqrt(dk)) v @ Wo

Sharding: data-parallel over batch (4) x tensor-parallel over heads (2
groups of 8) = 8 cores. Core (b, hg) projects q/k/v for its 8 heads over
the FULL T of batch b (no duplicated projection work), runs attention for
those heads, transposes the attention output on the PE, and pair-exchanges
the transposed outputs via per-head-pair AllGathers so both cores assemble
the full [QK, T] attention output. The output projection is sharded by
D-half (core (b, hg) emits out[b, :, hg*1024:(hg+1)*1024]), which keeps
the program rank-symmetric: every core reads BOTH AllGather slots.

Device layouts (bf16 compute, fp32 PSUM accumulation):
  xqT/xkT/xvT [D, T] host-transposed full batch (D on partitions)
  wq/wk/wv [D, 512] my head-half columns; wo [1024, 512..] my D-half cols
  kT/qT [128, 4(head-pair), T]: head pair g rows at partitions, T free
  vaug [128, T/128, 8, 65]: v natural per Tk chunk + ones column
  S^T tile [Tk 128, Tq 512] = kT-chunk.T @ qT block (K=dk=64)
  P^T = exp(S^T/8) (scores bounded ~+-5 -> no max-subtraction pass)
  PV natural: pav [Tq-chunk 128, 65] = P^T-chunk.T @ vaug accumulated over
    Tk chunks (M=128 full PE width, 2x fewer cycles than transposed PV);
    col 64 = softmax denominators -> per-partition reciprocal multiply
  avn [128, T/128, 128] per head-pair -> 16 PE transposes -> avT [128, T]
  aoT [128, 8, T]: global QK chunk c=r*4+g from AllGather slot r
  out rows = aoT.T @ wo accumulated over the 8 QK chunks (last head-pair's
    chunks accumulated last to hide the final exchange).
"""

import os

import ml_dtypes
import numpy as np

import concourse.bass as bass
from concourse import bacc, masks
import concourse.mybir as mybir
import concourse.tile as tile
from concourse.bass_utils import run_bass_kernel_spmd

BF16 = ml_dtypes.bfloat16
BF = mybir.dt.bfloat16
FP32 = mybir.dt.float32

B = 4
T = 2048
D = 2048
H = 16
DKH = 64
QK = H * DKH      # 1024
QKH = QK // 2     # 512 my head-half
NG = 4            # head pairs per core
KD = D // 128     # 16 contraction chunks for projections
TC = T // 128     # 16 Tk/Tq chunks
NCORES = 8
CC_SHARED = False  # pair-shared HBM AllGather output unsupported for 2-core groups

LAST = None  # BassKernelResults of the most recent run (for test harness)

_cache = {}


def _install_ntff_shim():
    """Provide antenv.axon_hooks + disable artifact upload so that
    run_bass_kernel_spmd(trace=True) can profile under axon in this image."""
    import sys
    import types

    try:
        import antenv.axon_hooks  # noqa: F401
    except ImportError:
        import antenv
        mod = types.ModuleType("antenv.axon_hooks")
        _h = [None]
        mod.set_axon_ntff_profile_hook = lambda h: _h.__setitem__(0, h)
        mod.get_axon_ntff_profile_hook = lambda: _h[0]
        sys.modules["antenv.axon_hooks"] = mod
        antenv.axon_hooks = mod
        try:
            from trn_agent_boot.trn_boot import _ntff_profile_via_ctypes
            mod.set_axon_ntff_profile_hook(
                _ntff_profile_via_ctypes("/opt/axon/libaxon_pjrt.so"))
        except Exception as e:
            print(f"ntff hook registration failed: {e}")
    try:
        import concourse.bass_utils as bu
        bu.upload_artifacts = lambda tmpdir: f"local:{tmpdir}"
    except Exception:
        pass


def _emit(tc, xqT, xkT, xvT, wq, wk, wv, wo, out, cc_in, cc_out):
    nc = tc.nc
    exp_f = mybir.ActivationFunctionType.Exp
    pairs = [[0, 1], [2, 3], [4, 5], [6, 7]]

    with tc.tile_pool(name="persist", bufs=1) as persist:
        kT = persist.tile([128, NG, T], BF, tag="kT")
        qT = persist.tile([128, NG, T], BF, tag="qT")
        vaug = persist.tile([128, TC, 8, DKH + 1], BF, tag="vaug")
        aoT = persist.tile([128, 8, T], BF, tag="aoT")
        ident = persist.tile([128, 128], BF, tag="ident")
        wo_sb = persist.tile([128, 2, 8, 512], BF, tag="wo")
        nc.vector.memset(vaug[:, :, :, DKH:DKH + 1], 1.0)
        masks.make_identity(nc, ident)
        # wo prefetch (2MB) so phase 3 has no DMA dependency
        for nb in range(2):
            for k in range(8):
                nc.sync.dma_start(
                    out=wo_sb[:, nb, k, :],
                    in_=wo[k * 128:(k + 1) * 128, nb * 512:(nb + 1) * 512])

        # ---- phase 1: q/k/v projections (full T, my 8 heads) ----
        with (
            nc.named_scope("p1_qkvproj"),
            tc.tile_pool(name="wkv", bufs=1) as w_pool,
            tc.tile_pool(name="xk", bufs=17) as xk_pool,
            tc.tile_pool(name="xv", bufs=17) as xv_pool,
            tc.tile_pool(name="xq", bufs=17) as xq_pool,
            tc.tile_pool(name="ps1", bufs=6, space="PSUM") as ps1,
        ):
            wk_sb = w_pool.tile([128, KD, QKH], BF, tag="wk")
            wv_sb = w_pool.tile([128, KD, QKH], BF, tag="wv")
            wq_sb = w_pool.tile([128, KD, QKH], BF, tag="wq")

            for nb in range(T // 512):  # 4 T-blocks
                xk_t = []
                xv_t = []
                xq_t = []
                for k in range(KD):
                    xkt = xk_pool.tile([128, 512], BF, tag="xk")
                    if nb == 0:
                        nc.sync.dma_start(out=wk_sb[:, k, :], in_=wk[k * 128:(k + 1) * 128, :])
                    nc.sync.dma_start(out=xkt, in_=xkT[k * 128:(k + 1) * 128, nb * 512:(nb + 1) * 512])
                    xk_t.append(xkt)
                for k in range(KD):
                    xvt = xv_pool.tile([128, 512], BF, tag="xv")
                    if nb == 0:
                        nc.sync.dma_start(out=wv_sb[:, k, :], in_=wv[k * 128:(k + 1) * 128, :])
                    nc.sync.dma_start(out=xvt, in_=xvT[k * 128:(k + 1) * 128, nb * 512:(nb + 1) * 512])
                    xv_t.append(xvt)
                for k in range(KD):
                    xqt = xq_pool.tile([128, 512], BF, tag="xq")
                    if nb == 0:
                        nc.sync.dma_start(out=wq_sb[:, k, :], in_=wq[k * 128:(k + 1) * 128, :])
                    nc.sync.dma_start(out=xqt, in_=xqT[k * 128:(k + 1) * 128, nb * 512:(nb + 1) * 512])
                    xq_t.append(xqt)

                # kT[g-slice, this T block] = wk_slice.T @ xk
                for g in range(NG):
                    ps = ps1.tile([128, 512], FP32, tag="ps1")
                    for k in range(KD):
                        nc.tensor.matmul(
                            ps, wk_sb[:, k, g * 128:(g + 1) * 128], xk_t[k],
                            start=(k == 0), stop=(k == KD - 1))
                    nc.vector.tensor_copy(out=kT[:, g, nb * 512:(nb + 1) * 512], in_=ps)
                # v natural [T-chunk rows, my 512 cols]
                for msl in range(4):
                    ms = nb * 4 + msl
                    ps = ps1.tile([128, 512], FP32, tag="ps1")
                    for k in range(KD):
                        nc.tensor.matmul(
                            ps, xv_t[k][:, msl * 128:(msl + 1) * 128], wv_sb[:, k, :],
                            start=(k == 0), stop=(k == KD - 1))
                    nc.vector.tensor_copy(
                        out=vaug[:, ms, :, 0:DKH],
                        in_=ps.rearrange("p (h d) -> p h d", d=DKH))
                # qT
                for g in range(NG):
                    ps = ps1.tile([128, 512], FP32, tag="ps1")
                    for k in range(KD):
                        nc.tensor.matmul(
                            ps, wq_sb[:, k, g * 128:(g + 1) * 128], xq_t[k],
                            start=(k == 0), stop=(k == KD - 1))
                    nc.vector.tensor_copy(out=qT[:, g, nb * 512:(nb + 1) * 512], in_=ps)

        # ---- phase 2: attention per head pair + pair exchange ----
        with (
            nc.named_scope("p2_attn"),
            tc.tile_pool(name="pt", bufs=10) as pt_pool,
            tc.tile_pool(name="avn", bufs=2) as avn_pool,
            tc.tile_pool(name="avT", bufs=2) as avT_pool,
            tc.tile_pool(name="dv", bufs=6) as dv_pool,
            tc.tile_pool(name="pss", bufs=2, space="PSUM") as ps_s,
            tc.tile_pool(name="pav", bufs=4, space="PSUM") as ps_av,
        ):
            for g in range(NG):
                avn = avn_pool.tile([128, TC, 128], BF, tag="avn")
                for hp in range(2):
                    h = 2 * g + hp
                    pk = slice(hp * 64, (hp + 1) * 64)
                    for n in range(T // 512):  # 4 Tq blocks
                        qs = qT[pk, g, n * 512:(n + 1) * 512]
                        pts = []
                        for t in range(8):  # pairs of Tk chunks
                            pss = ps_s.tile([128, 1024], FP32, tag="pss")
                            for c2 in range(2):
                                c = 2 * t + c2
                                nc.tensor.matmul(
                                    pss[:, c2 * 512:(c2 + 1) * 512],
                                    kT[pk, g, c * 128:(c + 1) * 128],
                                    qs, start=True, stop=True)
                            ptt = pt_pool.tile([128, 1024], BF, tag="pt")
                            nc.scalar.activation(out=ptt, in_=pss, func=exp_f, scale=0.125)
                            pts.append(ptt)
                        # PV natural, Tk-chunk-major across the 4 Tq chunks
                        pavs = [ps_av.tile([128, 512], FP32, tag="pav", name=f"pav{i}")
                                for i in range(4)]
                        for c in range(TC):
                            for tqc in range(4):
                                nc.tensor.matmul(
                                    pavs[tqc][:, 0:DKH + 1],
                                    pts[c // 2][:, (c % 2) * 512 + tqc * 128:(c % 2) * 512 + (tqc + 1) * 128],
                                    vaug[:, c, h, :],
                                    start=(c == 0), stop=(c == TC - 1))
                        for tqc in range(4):
                            linv = dv_pool.tile([128, 1], FP32, tag="linv")
                            nc.vector.reciprocal(out=linv, in_=pavs[tqc][:, DKH:DKH + 1])
                            nc.vector.tensor_scalar_mul(
                                avn[:, n * 4 + tqc, pk],
                                pavs[tqc][:, 0:DKH], linv)
                # transpose head pair g: avn [T-chunk, 128] -> avT [128, T]
                avTg = avT_pool.tile([128, T], BF, tag="avT")
                for tcx in range(TC):
                    tp = ps_s.tile([128, 128], BF, tag="pss")
                    nc.tensor.transpose(tp, avn[:, tcx, :], ident)
                    nc.vector.tensor_copy(
                        out=avTg[:, tcx * 128:(tcx + 1) * 128], in_=tp)
                # pair exchange: stage -> AllGather -> scatter both slots
                nc.sync.dma_start(out=cc_in[g], in_=avTg)
                nc.gpsimd.collective_compute(
                    "AllGather", mybir.AluOpType.bypass,
                    replica_groups=pairs,
                    ins=[cc_in[g]], outs=[cc_out[g]])
                for r in range(2):
                    nc.sync.dma_start(out=aoT[:, r * 4 + g, :], in_=cc_out[g, r])

        # ---- phase 3: output projection (my D-half, full T) ----
        with (
            nc.named_scope("p3_oproj"),
            tc.tile_pool(name="ostg", bufs=6) as o_pool,
            tc.tile_pool(name="pso", bufs=6, space="PSUM") as ps_o,
        ):
            # PE warmth bridge across the final exchange gap: reads aoT
            # chunk 0 (available after the first head-pair's exchange).
            for i in range(8):
                ps = ps_o.tile([128, 512], FP32, tag="pso")
                nc.tensor.matmul(
                    ps, aoT[:, 0, (i * 128) % T:((i * 128) % T) + 128],
                    aoT[:, 0, 0:512], start=True, stop=True)
            kk_order = [0, 4, 1, 5, 2, 6, 3, 7]  # last head-pair's chunks last
            for nb in range(2):  # D-half blocks of 512
                for m in range(TC):  # 16 T chunks
                    ps = ps_o.tile([128, 512], FP32, tag="pso")
                    for i, kk in enumerate(kk_order):
                        nc.tensor.matmul(
                            ps, aoT[:, kk, m * 128:(m + 1) * 128],
                            wo_sb[:, nb, kk, :],
                            start=(i == 0), stop=(i == 7))
                    stg = o_pool.tile([128, 512], FP32, tag="ostg")
                    nc.vector.tensor_copy(out=stg, in_=ps)
                    nc.sync.dma_start(
                        out=out[m * 128:(m + 1) * 128, nb * 512:(nb + 1) * 512], in_=stg)


def _build():
    if "nc" in _cache:
        return _cache["nc"]
    nc = bacc.Bacc("TRN2", target_bir_lowering=False, debug=False, num_devices=NCORES)
    xqT = nc.dram_tensor("xqT", [D, T], BF, kind="ExternalInput").ap()
    xkT = nc.dram_tensor("xkT", [D, T], BF, kind="ExternalInput").ap()
    xvT = nc.dram_tensor("xvT", [D, T], BF, kind="ExternalInput").ap()
    wq = nc.dram_tensor("wq", [D, QKH], BF, kind="ExternalInput").ap()
    wk = nc.dram_tensor("wk", [D, QKH], BF, kind="ExternalInput").ap()
    wv = nc.dram_tensor("wv", [D, QKH], BF, kind="ExternalInput").ap()
    wo = nc.dram_tensor("wo", [QK, D // 2], BF, kind="ExternalInput").ap()
    out = nc.dram_tensor("out", [T, D // 2], mybir.dt.float32, kind="ExternalOutput").ap()
    cc_in = nc.dram_tensor("cc_in", [NG, 128, T], BF, kind="Internal").ap()
    cc_out = nc.dram_tensor(
        "cc_out", [NG, 2, 128, T], BF, kind="Internal",
        addr_space="Shared" if CC_SHARED else "Local").ap()
    with tile.TileContext(nc) as tc:
        _emit(tc, xqT, xkT, xvT, wq, wk, wv, wo, out, cc_in, cc_out)
    nc.compile()
    _cache["nc"] = nc
    return nc


def kernel(**inputs):
    global LAST
    Q = np.asarray(inputs["Q"], dtype=np.float32)
    K = np.asarray(inputs["K"], dtype=np.float32)
    V = np.asarray(inputs["V"], dtype=np.float32)
    wq_f = np.asarray(inputs["Wq"], dtype=np.float32)
    wk_f = np.asarray(inputs["Wk"], dtype=np.float32)
    wv_f = np.asarray(inputs["Wv"], dtype=np.float32)
    wo_f = np.asarray(inputs["Wo"], dtype=np.float32)

    nc = _build()
    xb = []
    for b in range(B):
        xb.append({
            "xqT": np.ascontiguousarray(Q[b].T).astype(BF16),
            "xkT": np.ascontiguousarray(K[b].T).astype(BF16),
            "xvT": np.ascontiguousarray(V[b].T).astype(BF16),
        })
    wh = []
    for hg in range(2):
        sl = slice(hg * QKH, (hg + 1) * QKH)
        wh.append({
            "wq": np.ascontiguousarray(wq_f[:, sl]).astype(BF16),
            "wk": np.ascontiguousarray(wk_f[:, sl]).astype(BF16),
            "wv": np.ascontiguousarray(wv_f[:, sl]).astype(BF16),
            "wo": np.ascontiguousarray(wo_f[:, hg * 1024:(hg + 1) * 1024]).astype(BF16),
        })
    in_maps = []
    for core in range(NCORES):
        b, hg = core // 2, core % 2
        in_maps.append({**xb[b], **wh[hg]})

    want_trace = bool(os.environ.get("BASS_TRACE"))
    if want_trace:
        _install_ntff_shim()
        try:
            res = run_bass_kernel_spmd(
                nc, in_maps, core_ids=list(range(NCORES)), trace=True)
        except Exception as e:  # profiling infra missing -> still get results
            print(f"trace run failed ({type(e).__name__}: {e}); retrying untraced")
            res = run_bass_kernel_spmd(nc, in_maps, core_ids=list(range(NCORES)))
    else:
        res = run_bass_kernel_spmd(nc, in_maps, core_ids=list(range(NCORES)))
    LAST = res
    if res.exec_time_ns is not None:
        print(f"HW exec time: {res.exec_time_ns} ns")

    out = np.empty((B, T, D), np.float32)
    for core in range(NCORES):
        b, hg = core // 2, core % 2
        out[b, :, hg * 1024:(hg + 1) * 1024] = res.results[core]["out"]
    return out
